# revision 2
# baseline (speedup 1.0000x reference)
"""GATv2 layer kernel for 8 Trainium2 NeuronCores.

Math (reference is a GATv2 layer with N=8192 nodes, 128 in / 64 out feats):
    Wh  = mole_out @ W                      [N, 64]
    lr  = leakyrelu(Wh, 0.2)
    s1  = lr @ b[:64];  s2 = lr @ b[64:]
    e   = s1[:, None] + s2[None, :]         (masked by adj, row softmax)
    out = elu(softmax(e) @ Wh)

Key identity: s1[r] is constant along a softmax row, so it cancels:
    att[r, j] = adj[r, j] * exp(s2[j]) / sum_j adj[r, j] * exp(s2[j])
Let ev = exp(s2), G = diag(ev) @ Wh, H2 = [G | ev]  ([N, 65]).
Then raw[r, :] = sum_j adj[r, j] * H2[j, :]  and
    out[r, f] = elu(raw[r, f] / raw[r, 64]).
The whole attention collapses into one masked matmul against adj.

Sharding: rows (destination nodes) across 8 cores, 1024 rows each.  Each
core receives its adj slice TRANSPOSED, contiguous, and re-encoded as
fp8_e4m3 {0.0, 1.0} ([8192(j), 1024(r)], 8MB — 4x less HBM traffic than
the int32 original; 0/1 is exact in fp8).  W / b / mole_out replicated
(mole transposed + cast fp16 on host).  Per-core device work:
  - pre-pass: Wh (+ a folded 0.2*(W@b2) column), fused relu*b2 (one DVE
      STT op), s2, ev, H2 = [ev*Wh | ev] in fp16
  - main: stream adjT fp8 in [128, jpd*1024] tiles (1-2MB HWDGE loads),
      feed the PE DIRECTLY (no cast):
      psum[sb] [65, 512] += H2[jc](fp16).T @ adj_tile(fp8)[:, ...]
  - epilogue: divide by the ev-sum row, elu, store out^T [64, 1024].
Output is assembled on host by stacking the 8 row blocks.
"""

import numpy as np
import ml_dtypes

import concourse.bacc as bacc
import concourse.mybir as mybir
import concourse.tile as tile
from concourse.bass_utils import run_bass_kernel_spmd

N = 8192          # nodes
C = 128           # input features
F = 64            # output features
NCORES = 8
RPC = N // NCORES  # rows (destination nodes) per core: 1024
ALPHA = 0.2

f32 = mybir.dt.float32
f16 = mybir.dt.float16
fp8 = mybir.dt.float8e4
AF = mybir.ActivationFunctionType
ALU = mybir.AluOpType
FP8_NP = ml_dtypes.float8_e4m3


def _emit(tc, n, rpc, repeat=1, abf_bufs=3, jpd=8, no_pre=False, epi_bufs=2,
          mole_splits=8):
    """Emit the per-core program. n = total nodes, rpc = rows per core.

    repeat > 1 re-streams the whole adj pass that many times (psum restarts
    each pass, so outputs are unchanged) — used only to measure the
    steady-state main-loop time as a slope over repeat.
    """
    nc = tc.nc
    jt = n // 128          # number of j-chunks
    G = 4                  # Wh chunks per pre-pass group
    ng = jt // G
    nsb = rpc // 512       # superblocks of 512 destination rows
    F1 = F + 1

    adjT = nc.dram_tensor("adjT", [n, rpc], fp8, kind="ExternalInput").ap()
    moleT = nc.dram_tensor("moleT", [C, n], f16, kind="ExternalInput").ap()
    Waug = nc.dram_tensor("Waug", [C, F1], f16, kind="ExternalInput").ap()
    b2r = nc.dram_tensor("b2r4", [128, G * F], f32, kind="ExternalInput").ap()
    outT = nc.dram_tensor("outT", [F, rpc], f32, kind="ExternalOutput").ap()

    with (
        tc.tile_pool(name="const", bufs=1) as const,
        tc.tile_pool(name="preps", bufs=4, space="PSUM") as pre_ps,
        tc.tile_pool(name="sml", bufs=3) as sml,
        tc.tile_pool(name="abf", bufs=abf_bufs) as abfp,
        tc.tile_pool(name="mainps", bufs=1, space="PSUM") as main_ps,
        tc.tile_pool(name="bcps", bufs=2, space="PSUM") as bc_ps,
        tc.tile_pool(name="epi", bufs=epi_bufs) as epi,
    ):
        moleT_sb = const.tile([C, n], f16)
        if no_pre:  # timing-model experiment only: skip the mole load too
            nc.gpsimd.memset(moleT_sb[:, 0:128], 0.0)
        else:
            # pre-pass inputs ride the ACT HWDGE ring so the adj stream
            # (on the SP ring) can't queue-block them
            for s in range(mole_splits):
                sl = slice(s * (n // mole_splits), (s + 1) * (n // mole_splits))
                nc.scalar.dma_start(moleT_sb[:, sl], moleT[:, sl])
        W_sb = const.tile([C, F1], f16)
        nc.scalar.dma_start(W_sb[:], Waug)
        b2_sb = const.tile([128, G * F], f32)
        nc.scalar.dma_start(b2_sb[:], b2r)
        H2 = const.tile([128, jt * F1], f16)
        ones_sb = const.tile([1, F], f32)
        nc.gpsimd.memset(ones_sb[:], 1.0)

        h2v = H2[:].rearrange("p (c f) -> p c f", f=F1)

        # ---- pre-pass: Wh, s2, ev, H2 = [ev*Wh | ev] ----
        if no_pre:  # timing-model experiment only: skip H2 construction
            nc.gpsimd.memset(H2[:], 0.0)
        for g in range(0 if no_pre else ng):
            ps = pre_ps.tile([128, G * F1], f32)
            for q in range(G):
                cc = g * G + q
                # [128(i), 65] = moleT[:, i-chunk].T @ [W | 0.2*W@b2]
                nc.tensor.matmul(
                    ps[:, q * F1:(q + 1) * F1],
                    lhsT=moleT_sb[:, cc * 128:(cc + 1) * 128],
                    rhs=W_sb[:],
                    start=True,
                    stop=True,
                )
            ps3 = ps[:].rearrange("p (g f) -> p g f", f=F1)
            wh4 = ps3[:, :, 0:F]          # [128, G, 64] Wh values
            sw4 = ps3[:, :, F:F1]         # [128, G, 1]  0.2*(Wh@b2)
            m4 = sml.tile([128, G * F], f32, tag="m4")
            # m4 = relu(Wh) * (0.8*b2), fused in one DVE op
            nc.vector.scalar_tensor_tensor(
                m4[:].rearrange("p (g f) -> p g f", f=F), wh4, 0.0, b2_sb[:].rearrange("p (g f) -> p g f", f=F),
                op0=ALU.max, op1=ALU.mult,
            )
            sr4 = sml.tile([128, G], f32, tag="sr4")
            nc.vector.tensor_reduce(
                sr4[:], m4[:].rearrange("p (g f) -> p g f", f=F),
                axis=mybir.AxisListType.X, op=ALU.add,
            )
            s24 = sml.tile([128, G], f32, tag="s24")
            nc.vector.tensor_add(s24[:], sw4, sr4[:])      # s2 = 0.2*sw + 0.8*sr
            ev4 = sml.tile([128, G], f32, tag="ev4")
            nc.scalar.activation(ev4[:], s24[:], AF.Exp)
            for q in range(G):
                cc = g * G + q
                nc.scalar.mul(
                    h2v[:, cc:cc + 1, 0:F], ps3[:, q:q + 1, 0:F], ev4[:, q:q + 1]
                )
            nc.vector.tensor_copy(h2v[:, g * G:(g + 1) * G, F:F1], ev4[:])

        # ---- main: psum[sb] [65, 512] += H2[jc](f16).T @ adjT(fp8) ----
        pss = [
            main_ps.tile([F1, 512], f32, name=f"mps{sb}", tag=f"mps{sb}")
            for sb in range(nsb)
        ]
        # adjT rows viewed as [jt, 128, rpc]; one DMA may carry several
        # chunks.  The first few loads are narrow so the DMA->PE pipeline
        # primes quickly, then steady state uses jpd-wide tiles.
        adjT3 = adjT.rearrange("(c p) r -> c p r", p=128)
        prime = (1, 1, 2)
        widths = []
        if jt > sum(prime) and (jt - sum(prime)) % jpd == 0:
            widths = list(prime)
        while sum(widths) < jt:
            widths.append(jpd)
        for rep in range(repeat):
            c0 = 0
            for jd, w in enumerate(widths):
                src = adjT3[c0:c0 + w, :, :].rearrange("c p r -> p c r")
                abf = abfp.tile([128, w * rpc], fp8, name="abf", tag="abf")
                nc.sync.dma_start(
                    abf[:].rearrange("p (c r) -> p c r", c=w), src
                )
                for h in range(w):
                    jc = c0 + h
                    for sb in range(nsb):
                        nc.tensor.matmul(
                            pss[sb][:],
                            lhsT=H2[:, jc * F1:(jc + 1) * F1],
                            rhs=abf[:, h * rpc + sb * 512:h * rpc + (sb + 1) * 512],
                            start=(jc == 0),
                            stop=(jc == jt - 1),
                        )
                c0 += w

        # ---- epilogue: out = elu(num / den), stored transposed ----
        for sb in range(nsb):
            ps = pss[sb]
            rec = epi.tile([1, 512], f32, tag="rec")
            nc.vector.reciprocal(rec[:], ps[F:F1, :])
            bc = bc_ps.tile([F, 512], f32)
            nc.tensor.matmul(bc[:], lhsT=ones_sb[:], rhs=rec[:], start=True, stop=True)
            bc_sb = epi.tile([F, 512], f32, tag="bc")
            nc.vector.tensor_copy(bc_sb[:], bc[:])
            x = epi.tile([F, 512], f32, tag="x")
            nc.vector.tensor_mul(x[:], ps[0:F, :], bc_sb[:])
            mneg = epi.tile([F, 512], f32, tag="mneg")
            nc.vector.tensor_scalar_min(mneg[:], x[:], 0.0)
            e = epi.tile([F, 512], f32, tag="e")
            nc.scalar.activation(e[:], mneg[:], AF.Exp)
            r = epi.tile([F, 512], f32, tag="r")
            nc.scalar.activation(r[:], x[:], AF.Relu)
            o = epi.tile([F, 512], f32, tag="o")
            # o = (e + (-1)) + r  == elu(x)
            nc.vector.scalar_tensor_tensor(
                o[:], e[:], -1.0, r[:], op0=ALU.add, op1=ALU.add
            )
            nc.sync.dma_start(outT[:, sb * 512:(sb + 1) * 512], o[:])


_CACHE = {}


def _build(n=N, rpc=RPC, repeat=1, abf_bufs=3, jpd=8, swdge_queues=1,
           no_pre=False, epi_bufs=2, mole_splits=8):
    key = (n, rpc, repeat, abf_bufs, jpd, swdge_queues, no_pre, epi_bufs,
           mole_splits)
    if key not in _CACHE:
        nc = bacc.Bacc(
            "TRN2", target_bir_lowering=False, debug=False, num_devices=NCORES,
            num_swdge_queues=swdge_queues,
        )
        with tile.TileContext(nc) as tc:
            _emit(tc, n, rpc, repeat, abf_bufs, jpd, no_pre, epi_bufs,
                  mole_splits)
        nc.compile()
        _CACHE[key] = nc
    return _CACHE[key]


def _host_prep(mole_out, adj, W, b, n=N, rpc=RPC, ncores=NCORES):
    mole_out = np.asarray(mole_out, dtype=np.float32)
    adj = np.asarray(adj)
    W = np.asarray(W, dtype=np.float32)
    b = np.asarray(b, dtype=np.float32)
    b2 = b[F:]
    moleT = np.ascontiguousarray(mole_out.T.astype(np.float16))  # [128, n]
    Waug = np.concatenate([W, (ALPHA * (W @ b2))[:, None]], axis=1)
    Waug = np.ascontiguousarray(Waug.astype(np.float16))         # [128, 65]
    b2r4 = np.tile(((1.0 - ALPHA) * b2).astype(np.float32), (128, 4))
    b2r4 = np.ascontiguousarray(b2r4)                            # [128, 256]
    # adjacency as fp8 {0.0, 1.0}: 1.0 in e4m3 is byte 0x38
    adj8 = (np.asarray(adj, dtype=np.uint8) * np.uint8(0x38)).view(FP8_NP)
    in_maps = []
    for k in range(ncores):
        adjTk = np.ascontiguousarray(adj8[k * rpc:(k + 1) * rpc, :].T)
        in_maps.append(
            {"adjT": adjTk, "moleT": moleT, "Waug": Waug, "b2r4": b2r4}
        )
    return in_maps


def _run(inputs, trace=False, **kw):
    nc = _build()
    in_maps = _host_prep(**inputs)
    res = run_bass_kernel_spmd(
        nc, in_maps, core_ids=list(range(NCORES)), trace=trace, **kw
    )
    out = np.concatenate([r["outT"].T for r in res.results], axis=0)
    return np.ascontiguousarray(out, dtype=np.float32), res


def kernel(mole_out, adj, W, b):
    out, _ = _run(dict(mole_out=mole_out, adj=adj, W=W, b=b))
    return out


# revision 22
# speedup vs baseline: 555.3623x; 555.3623x over previous
"""GATv2 layer kernel for 8 Trainium2 NeuronCores.

Math (reference is a GATv2 layer with N=8192 nodes, 128 in / 64 out feats):
    Wh  = mole_out @ W                      [N, 64]
    lr  = leakyrelu(Wh, 0.2)
    s1  = lr @ b[:64];  s2 = lr @ b[64:]
    e   = s1[:, None] + s2[None, :]         (masked by adj, row softmax)
    out = elu(softmax(e) @ Wh)

Key identity: s1[r] is constant along a softmax row, so it cancels:
    att[r, j] = adj[r, j] * exp(s2[j]) / sum_j adj[r, j] * exp(s2[j])
Let ev = exp(s2), G = diag(ev) @ Wh, H2 = [G | ev]  ([N, 65]).
Then raw[r, :] = sum_j adj[r, j] * H2[j, :]  and
    out[r, f] = elu(raw[r, f] / raw[r, 64]).
The whole attention collapses into one masked matmul against adj.

Sharding: rows (destination nodes) across 8 cores, 1024 rows each.  Each
core receives its adj slice TRANSPOSED, host-packed into [4, 128,
16*1024] fp8_e4m3 {0.0, 1.0} tiles (8MB vs 32MB int32; 0/1 is exact in
fp8; each partition reads one 16KB contiguous run per DMA).  W / b /
mole_out replicated (mole transposed + cast fp16 on host).

Per-core device schedule (one-shot path) — the pre-pass is emitted
INTERLEAVED with the main matmuls so the PE's in-order queue never
stalls main work behind pre-pass work that waits on later mole splits:

  DMA(SP ring):  adj tile 0..3 (2MB each, issued upfront)
  DMA(ACT ring): W, b2, mole split 0..3 (0.5MB each)
  per section s in 0..3:
    pre:  16 matmuls Wh -> psum, DVE chain (fused relu*b2, reduce, exp)
          -> H2 section s = [ev*Wh | ev]  (fp8 for DoubleRow, else fp16)
    main: 16 DR matmuls  psum[sb] += H2[pair].T @ adj tile s
  epilogue: out = elu(num/den) per superblock, one [64, 1024] store.

Main matmuls use fp8 DoubleRow (2 j-chunks per pass, 2 MACs/cell/cycle)
against fp8 H2; rel err vs the fp32 reference is 1.7e-2 (fp8 H2
quantisation; gate is 2e-2).  dr=False falls back to fp16 H2 normal
matmuls (rel err 2.5e-4) at ~1.7x the PE time.
"""

import contextlib

import numpy as np
import ml_dtypes

import concourse.bacc as bacc
import concourse.mybir as mybir
import concourse.tile as tile
from concourse.bass_utils import run_bass_kernel_spmd


def _nullctx():
    return contextlib.nullcontext()


N = 8192          # nodes
C = 128           # input features
F = 64            # output features
NCORES = 8
RPC = N // NCORES  # rows (destination nodes) per core: 1024
ALPHA = 0.2

f32 = mybir.dt.float32
f16 = mybir.dt.float16
fp8 = mybir.dt.float8e4
AF = mybir.ActivationFunctionType
ALU = mybir.AluOpType
FP8_NP = ml_dtypes.float8_e4m3


def _emit(tc, n, rpc, repeat=1, abf_bufs=4, jpd=16, no_pre=False, epi_bufs=2,
          mole_splits=4, packed=True, same_w=False, nop=False, dr=True,
          no_mole=False, full=1, hw_repeat=1, hw_full=1, probe=None,
          dma_alt=False, interleave=True, dma_split=2):
    """Emit the per-core program. n = total nodes, rpc = rows per core.

    interleave=True (the shipping path) emits pre-pass section s followed
    by that section's main matmuls; requires hw_repeat == 1 and no probe.
    repeat / hw_repeat re-stream the main pass (psum restarts per pass) to
    measure steady-state slope; hw_full loops the WHOLE kernel via a
    hardware loop for one-shot-time measurement.
    same_w / nop / probe are timing-model probes only (wrong output).
    """
    nc = tc.nc
    jt = n // 128          # number of j-chunks
    G = 4                  # Wh chunks per pre-pass psum bank
    NSEC = 4               # pre-pass sections
    SEC = jt // NSEC       # j-chunks per section: 16
    nsb = rpc // 512       # superblocks of 512 destination rows
    F1 = F + 1
    if (hw_repeat > 1 or repeat > 1 or no_pre or same_w
            or probe in ("pe_only", "dma_only")):
        interleave = False
    if interleave:
        assert packed and jpd == SEC and dr is not None

    if nop:
        outT = nc.dram_tensor("outT", [F, rpc], f32, kind="ExternalOutput").ap()
        with tc.tile_pool(name="nop", bufs=1) as npool:
            z = npool.tile([F, rpc], f32)
            nc.gpsimd.memset(z[:, 0:1], 0.0)
            nc.sync.dma_start(outT[:, :], z[:])
        return

    if packed:
        adjTp = nc.dram_tensor(
            "adjT", [n // (128 * jpd), 128, jpd * rpc], fp8,
            kind="ExternalInput",
        ).ap()
    else:
        adjT = nc.dram_tensor("adjT", [n, rpc], fp8, kind="ExternalInput").ap()
    moleT = nc.dram_tensor("moleT", [C, n], f16, kind="ExternalInput").ap()
    Waug = nc.dram_tensor("Waug", [C, F1], f16, kind="ExternalInput").ap()
    b2r = nc.dram_tensor("b2r16", [128, SEC * F], f32,
                         kind="ExternalInput").ap()
    outT = nc.dram_tensor("outT", [F, rpc], f32, kind="ExternalOutput").ap()

    with (
        tc.tile_pool(name="const", bufs=1) as const,
        tc.tile_pool(name="preps", bufs=4, space="PSUM") as pre_ps,
        tc.tile_pool(name="sml", bufs=3) as sml,
        tc.tile_pool(name="abf", bufs=abf_bufs) as abfp,
        tc.tile_pool(name="mainps", bufs=1, space="PSUM") as main_ps,
        tc.tile_pool(name="bcps", bufs=2, space="PSUM") as bc_ps,
        tc.tile_pool(name="epi", bufs=epi_bufs) as epi,
    ):
        full_ctx = (
            tc.For_i(0, hw_full, name="fullrep") if hw_full > 1 else None
        )
        for _it in range(full):
          with full_ctx if full_ctx is not None else _nullctx():
            hdt = fp8 if dr else f16
            # DoubleRow weight APs need a 16-byte-aligned pair stride: pad
            # the per-chunk H2 stride from 65 to 80 fp8 elements
            F1P = 80 if dr else F1
            # constants first on the ACT ring so the pre-pass isn't gated
            # behind the mole stream
            W_sb = const.tile([C, F1], f16)
            nc.scalar.dma_start(W_sb[:], Waug)
            b2_sb = const.tile([128, SEC * F], f32)
            nc.scalar.dma_start(b2_sb[:], b2r)
            moleT_sb = const.tile([C, n], f16)
            if no_pre or no_mole:  # timing probes only: skip the mole load
                nc.gpsimd.memset(moleT_sb[:, 0:128], 0.0)
            else:
                # split 0 on the SP ring AHEAD of the adj tiles (ring FIFO
                # guarantees it lands first); splits 1..3 on the ACT ring
                # so they stream concurrently with adj
                for s in range(NSEC):
                    sl = slice(s * (n // NSEC), (s + 1) * (n // NSEC))
                    eng = nc.sync if (s == 0 and interleave) else nc.scalar
                    eng.dma_start(moleT_sb[:, sl], moleT[:, sl])
            H2 = const.tile([128, jt * F1P], hdt)
            ones_sb = const.tile([1, F], f32)
            nc.gpsimd.memset(ones_sb[:], 1.0)

            h2v = H2[:].rearrange("p (c f) -> p c f", f=F1P)

            pss = [
                main_ps.tile([F1, 512], f32, name=f"mps{sb}", tag=f"mps{sb}")
                for sb in range(nsb)
            ]

            def pre_section(s):
                """Wh, s2, ev for chunks [s*SEC, (s+1)*SEC) -> H2 section.

                DVE volume is the scarce resource: the relu*b2 STT reads Wh
                straight from psum (no staging copy), the reduce runs on its
                fp16 output, and the H2 G-part is per-group STTs of psum
                against a materialised ev expansion (broadcast APs combine
                only with contiguous outputs, so ev is expanded once).
                """
                swc = sml.tile([128, SEC], f32, tag="swc")
                m16 = sml.tile([128, SEC * F], f16, tag="m16")
                pss_g = []
                for g in range(SEC // G):
                    ps = pre_ps.tile([128, G * F1], f32)
                    pss_g.append(ps)
                    for q in range(G):
                        cc = s * SEC + g * G + q
                        # [128(i), 65] = moleT[:, chunk].T @ [W | 0.2*W@b2]
                        nc.tensor.matmul(
                            ps[:, q * F1:(q + 1) * F1],
                            lhsT=moleT_sb[:, cc * 128:(cc + 1) * 128],
                            rhs=W_sb[:],
                            start=True,
                            stop=True,
                        )
                    ps3 = ps[:].rearrange("p (g f) -> p g f", f=F1)
                    # m = relu(Wh) * (0.8*b2), psum-in, fp16 out
                    nc.vector.scalar_tensor_tensor(
                        m16[:, g * G * F:(g + 1) * G * F].rearrange(
                            "p (g f) -> p g f", f=F),
                        ps3[:, :, 0:F], 0.0,
                        b2_sb[:, g * G * F:(g + 1) * G * F].rearrange(
                            "p (g f) -> p g f", f=F),
                        op0=ALU.max, op1=ALU.mult,
                    )
                    nc.scalar.copy(swc[:, g * G:(g + 1) * G], ps3[:, :, F])
                sr = sml.tile([128, SEC], f32, tag="sr")
                nc.vector.tensor_reduce(
                    sr[:], m16[:].rearrange("p (c f) -> p c f", f=F),
                    axis=mybir.AxisListType.X, op=ALU.add,
                )
                s2s = sml.tile([128, SEC], f32, tag="s2s")
                nc.vector.tensor_add(s2s[:], swc[:], sr[:])
                ev = sml.tile([128, SEC], f32, tag="ev")
                nc.scalar.activation(ev[:], s2s[:], AF.Exp)
                evx = sml.tile([128, SEC * F], f16, tag="evx")
                nc.vector.tensor_copy(
                    evx[:].rearrange("p (c f) -> p c f", f=F),
                    ev[:].rearrange("p c -> p c ()").broadcast_to(
                        [128, SEC, F]),
                )
                evx3 = evx[:].rearrange("p (c f) -> p c f", f=F)
                for g in range(SEC // G):
                    ps3 = pss_g[g][:].rearrange("p (g f) -> p g f", f=F1)
                    # H2 G-part: (Wh * 1.0) * ev, psum-in, strided fp8 out
                    nc.vector.scalar_tensor_tensor(
                        h2v[:, s * SEC + g * G:s * SEC + (g + 1) * G, 0:F],
                        ps3[:, :, 0:F], 1.0,
                        evx3[:, g * G:(g + 1) * G, :],
                        op0=ALU.mult, op1=ALU.mult,
                    )
                nc.vector.tensor_copy(
                    h2v[:, s * SEC:(s + 1) * SEC, F:F1], ev[:]
                )

            def main_mms(abf, w, c0, sb_outer=False):
                """Main matmuls consuming adj chunks [c0, c0+w)."""
                if dr:
                    # fp8 DoubleRow: one matmul consumes a PAIR of j-chunks
                    abf3 = abf[:].rearrange("p (c r) -> p c r", c=w)
                    iters = ([(h, sb) for sb in range(nsb)
                              for h in range(0, w, 2)] if sb_outer else
                             [(h, sb) for h in range(0, w, 2)
                              for sb in range(nsb)])
                    for h, sb in iters:
                        if True:
                            jc = c0 + h
                            nc.tensor.matmul(
                                pss[sb][:],
                                lhsT=h2v[:, jc:jc + 2, 0:F1],
                                rhs=abf3[:, h:h + 2,
                                         sb * 512:(sb + 1) * 512],
                                start=(jc == 0),
                                stop=(jc == jt - 2),
                                perf_mode=mybir.MatmulPerfMode.DoubleRow,
                            )
                else:
                    for h in range(w):
                        jc = c0 + h
                        for sb in range(nsb):
                            nc.tensor.matmul(
                                pss[sb][:],
                                lhsT=h2v[:, 0:1, 0:F1] if same_w
                                else h2v[:, jc:jc + 1, 0:F1],
                                rhs=abf[:, h * rpc + sb * 512:
                                        h * rpc + (sb + 1) * 512],
                                start=(jc == 0),
                                stop=(jc == jt - 1),
                            )

            if interleave:
                # adj tile DMAs issued upfront on the SP ring (optionally in
                # dma_split sub-pieces so the first matmuls start earlier)
                abf_tiles = []
                for jd in range(NSEC):
                    abf = abfp.tile([128, jpd * rpc], fp8, name="abf",
                                    tag="abf")
                    a3 = abf[:].rearrange("p (c r) -> p c r", c=jpd)
                    for piece in range(dma_split):
                        cl = slice(piece * (jpd // dma_split),
                                   (piece + 1) * (jpd // dma_split))
                        nc.sync.dma_start(a3[:, cl, :], adjTp[jd, :, :]
                                          .rearrange("p (c r) -> p c r",
                                                     c=jpd)[:, cl, :])
                    abf_tiles.append(abf)
                for s in range(NSEC):
                    pre_section(s)
                    main_mms(abf_tiles[s], SEC, s * SEC,
                             sb_outer=(s == NSEC - 1))
            else:
                # measurement path: pre-pass fully first, then the main
                # pass (optionally looped) — matches the old structure
                if no_pre:
                    nc.gpsimd.memset(H2[:], 0.0)
                else:
                    for s in range(NSEC):
                        pre_section(s)
                adjT3 = (None if packed
                         else adjT.rearrange("(c p) r -> c p r", p=128))
                prime = (2, 2, 4) if dr else (1, 1, 2)
                widths = []
                if (not packed and jt > sum(prime)
                        and (jt - sum(prime)) % jpd == 0):
                    widths = list(prime)
                while sum(widths) < jt:
                    widths.append(jpd)
                if probe == "pe_only":
                    abf_c = const.tile([128, jpd * rpc], fp8, tag="abfc")
                    nc.vector.memset(abf_c[:], 0.0)
                rep_ctx = (
                    tc.For_i(0, hw_repeat, name="mainrep")
                    if hw_repeat > 1 else None
                )
                for rep in range(repeat):
                  with rep_ctx if rep_ctx is not None else _nullctx():
                    c0 = 0
                    for jd, w in enumerate(widths):
                        if packed:
                            src = adjTp[c0 // jpd]
                        else:
                            src = adjT3[c0:c0 + w, :, :].rearrange(
                                "c p r -> p c r")
                        if probe == "pe_only":
                            abf = abf_c
                        else:
                            abf = abfp.tile([128, w * rpc], fp8, name="abf",
                                            tag="abf")
                            dma_eng = (nc.scalar if (dma_alt and jd % 2)
                                       else nc.sync)
                            dma_eng.dma_start(
                                abf[:].rearrange("p (c r) -> p c r", c=w),
                                src,
                            )
                        if probe == "dma_only":
                            c0 += w
                            continue
                        main_mms(abf, w, c0)
                        c0 += w

            # ---- epilogue: out = elu(num / den), stored transposed ----
            if probe == "no_epi":
                for sb in range(nsb):
                    t = epi.tile([F1, 1], f32, tag=f"ne{sb}")
                    nc.vector.tensor_copy(t[:], pss[sb][:, 0:1])
                dz = epi.tile([F, rpc], f32, tag="dz")
                nc.gpsimd.memset(dz[:, 0:1], 0.0)
                nc.sync.dma_start(outT[:, :], dz[:])
                continue
            if probe:
                dz = epi.tile([F, rpc], f32, tag="dz")
                nc.gpsimd.memset(dz[:, 0:1], 0.0)
                nc.sync.dma_start(outT[:, :], dz[:])
                continue
            # per superblock: parallel num copy plus rec->bc (PE broadcast
            # of 1/den); the elu tail runs per superblock at [64, 512]
            # (full DVE/ACT rate) with both halves landing in one output
            # tile for a single 256KB store
            o = epi.tile([F, rpc], f32, tag="o")
            for sb in range(nsb):
                ps = pss[sb]
                numc = epi.tile([F, 512], f32, tag=f"numc{sb}")
                nc.vector.tensor_copy(numc[:], ps[0:F, :])
                rec = epi.tile([1, 512], f32, tag=f"rec{sb}")
                nc.vector.reciprocal(rec[:], ps[F:F1, :])
                bc = bc_ps.tile([F, 512], f32)
                nc.tensor.matmul(bc[:], lhsT=ones_sb[:], rhs=rec[:],
                                 start=True, stop=True)
                x = epi.tile([F, 512], f32, tag=f"x{sb}")
                nc.vector.tensor_mul(x[:], numc[:], bc[:])
                mneg = epi.tile([F, 512], f32, tag=f"mneg{sb}")
                nc.vector.tensor_scalar_min(mneg[:], x[:], 0.0)
                e = epi.tile([F, 512], f32, tag=f"e{sb}")
                nc.scalar.activation(e[:], mneg[:], AF.Exp)
                r = epi.tile([F, 512], f32, tag=f"r{sb}")
                nc.vector.tensor_scalar_max(r[:], x[:], 0.0)
                # o = (e + (-1)) + r  == elu(x)
                nc.vector.scalar_tensor_tensor(
                    o[:, sb * 512:(sb + 1) * 512], e[:], -1.0, r[:],
                    op0=ALU.add, op1=ALU.add,
                )
            nc.sync.dma_start(outT[:, :], o[:])


_CACHE = {}


def _build(n=N, rpc=RPC, repeat=1, abf_bufs=4, jpd=16, swdge_queues=1,
           no_pre=False, epi_bufs=2, mole_splits=4, packed=True,
           same_w=False, nop=False, dr=True, no_mole=False, full=1,
           hw_repeat=1, hw_full=1, probe=None, dma_alt=False,
           interleave=True, dma_split=2):
    key = (n, rpc, repeat, abf_bufs, jpd, swdge_queues, no_pre, epi_bufs,
           mole_splits, packed, same_w, nop, dr, no_mole, full, hw_repeat,
           hw_full, probe, dma_alt, interleave, dma_split)
    if key not in _CACHE:
        nc = bacc.Bacc(
            "TRN2", target_bir_lowering=False, debug=False, num_devices=NCORES,
            num_swdge_queues=swdge_queues,
        )
        with tile.TileContext(nc) as tc:
            _emit(tc, n, rpc, repeat, abf_bufs, jpd, no_pre, epi_bufs,
                  mole_splits, packed, same_w, nop, dr, no_mole, full,
                  hw_repeat, hw_full, probe, dma_alt, interleave, dma_split)
        nc.compile()
        _CACHE[key] = nc
    return _CACHE[key]


def _host_prep(mole_out, adj, W, b, n=N, rpc=RPC, ncores=NCORES,
               packed=True, jpd=16):
    mole_out = np.asarray(mole_out, dtype=np.float32)
    adj = np.asarray(adj)
    W = np.asarray(W, dtype=np.float32)
    b = np.asarray(b, dtype=np.float32)
    b2 = b[F:]
    moleT = np.ascontiguousarray(mole_out.T.astype(np.float16))  # [128, n]
    Waug = np.concatenate([W, (ALPHA * (W @ b2))[:, None]], axis=1)
    Waug = np.ascontiguousarray(Waug.astype(np.float16))         # [128, 65]
    b2r16 = np.tile(((1.0 - ALPHA) * b2).astype(np.float32), (128, 16))
    b2r16 = np.ascontiguousarray(b2r16)                          # [128, 1024]
    # adjacency as fp8 {0.0, 1.0}: 1.0 in e4m3 is byte 0x38
    adj8 = (np.asarray(adj, dtype=np.uint8) * np.uint8(0x38)).view(FP8_NP)
    in_maps = []
    for k in range(ncores):
        adjTk = np.ascontiguousarray(adj8[k * rpc:(k + 1) * rpc, :].T)
        if packed:
            adjTk = np.ascontiguousarray(
                adjTk.reshape(n // (128 * jpd), jpd, 128, rpc)
                .transpose(0, 2, 1, 3)
                .reshape(n // (128 * jpd), 128, jpd * rpc)
            )
        in_maps.append(
            {"adjT": adjTk, "moleT": moleT, "Waug": Waug, "b2r16": b2r16}
        )
    return in_maps


def _run(inputs, trace=False, build_kw=None, **kw):
    bkw = dict(build_kw or {})
    nc = _build(**bkw)
    in_maps = _host_prep(**inputs, packed=bkw.get("packed", True),
                         jpd=bkw.get("jpd", 16))
    res = run_bass_kernel_spmd(
        nc, in_maps, core_ids=list(range(NCORES)), trace=trace, **kw
    )
    out = np.concatenate([r["outT"].T for r in res.results], axis=0)
    return np.ascontiguousarray(out, dtype=np.float32), res


def kernel(mole_out, adj, W, b):
    out, _ = _run(dict(mole_out=mole_out, adj=adj, W=W, b=b))
    return out


# revision 23
# speedup vs baseline: 677.9181x; 1.2207x over previous
"""GATv2 layer kernel for 8 Trainium2 NeuronCores.

Math (reference is a GATv2 layer with N=8192 nodes, 128 in / 64 out feats):
    Wh  = mole_out @ W                      [N, 64]
    lr  = leakyrelu(Wh, 0.2)
    s1  = lr @ b[:64];  s2 = lr @ b[64:]
    e   = s1[:, None] + s2[None, :]         (masked by adj, row softmax)
    out = elu(softmax(e) @ Wh)

Key identity: s1[r] is constant along a softmax row, so it cancels:
    att[r, j] = adj[r, j] * exp(s2[j]) / sum_j adj[r, j] * exp(s2[j])
Let ev = exp(s2), G = diag(ev) @ Wh, H2 = [G | ev]  ([N, 65]).
Then raw[r, :] = sum_j adj[r, j] * H2[j, :]  and
    out[r, f] = elu(raw[r, f] / raw[r, 64]).
The whole attention collapses into one masked matmul against adj.

Sharding: rows (destination nodes) across 8 cores, 1024 rows each.  Each
core receives its adj slice TRANSPOSED, host-packed into [4, 128,
16*1024] fp8_e4m3 {0.0, 1.0} tiles (8MB vs 32MB int32; 0/1 is exact in
fp8; each partition reads one 16KB contiguous run per DMA).  W / b /
mole_out replicated (mole transposed + cast fp16 on host).

Per-core device schedule (one-shot path) — the pre-pass is emitted
INTERLEAVED with the main matmuls so the PE's in-order queue never
stalls main work behind pre-pass work that waits on later mole splits:

  DMA(SP ring):  adj tile 0..3 (2MB each, issued upfront)
  DMA(ACT ring): W, b2, mole split 0..3 (0.5MB each)
  per section s in 0..3:
    pre:  16 matmuls Wh -> psum, DVE chain (fused relu*b2, reduce, exp)
          -> H2 section s = [ev*Wh | ev]  (fp8 for DoubleRow, else fp16)
    main: 16 DR matmuls  psum[sb] += H2[pair].T @ adj tile s
  epilogue: out = elu(num/den) per superblock, one [64, 1024] store.

Main matmuls use fp8 DoubleRow (2 j-chunks per pass, 2 MACs/cell/cycle)
against fp8 H2; rel err vs the fp32 reference is 1.7e-2 (fp8 H2
quantisation; gate is 2e-2).  dr=False falls back to fp16 H2 normal
matmuls (rel err 2.5e-4) at ~1.7x the PE time.
"""

import contextlib

import numpy as np
import ml_dtypes

import concourse.bacc as bacc
import concourse.mybir as mybir
import concourse.tile as tile
from concourse.bass_utils import run_bass_kernel_spmd


def _nullctx():
    return contextlib.nullcontext()


N = 8192          # nodes
C = 128           # input features
F = 64            # output features
NCORES = 8
RPC = N // NCORES  # rows (destination nodes) per core: 1024
ALPHA = 0.2

f32 = mybir.dt.float32
f16 = mybir.dt.float16
fp8 = mybir.dt.float8e4
AF = mybir.ActivationFunctionType
ALU = mybir.AluOpType
FP8_NP = ml_dtypes.float8_e4m3


def _emit(tc, n, rpc, repeat=1, abf_bufs=4, jpd=16, no_pre=False, epi_bufs=2,
          mole_splits=4, packed=True, same_w=False, nop=False, dr=True,
          no_mole=False, full=1, hw_repeat=1, hw_full=1, probe=None,
          dma_alt=False, interleave=True, dma_split=2):
    """Emit the per-core program. n = total nodes, rpc = rows per core.

    interleave=True (the shipping path) emits pre-pass section s followed
    by that section's main matmuls; requires hw_repeat == 1 and no probe.
    repeat / hw_repeat re-stream the main pass (psum restarts per pass) to
    measure steady-state slope; hw_full loops the WHOLE kernel via a
    hardware loop for one-shot-time measurement.
    same_w / nop / probe are timing-model probes only (wrong output).
    """
    nc = tc.nc
    jt = n // 128          # number of j-chunks
    G = 4                  # Wh chunks per pre-pass psum bank
    NSEC = 4               # pre-pass sections
    SEC = jt // NSEC       # j-chunks per section: 16
    nsb = rpc // 512       # superblocks of 512 destination rows
    F1 = F + 1
    if (hw_repeat > 1 or repeat > 1 or no_pre or same_w
            or probe in ("pe_only", "dma_only")):
        interleave = False
    if interleave:
        assert packed and jpd == SEC and dr is not None

    if nop:
        outT = nc.dram_tensor("outT", [F, rpc], f32, kind="ExternalOutput").ap()
        with tc.tile_pool(name="nop", bufs=1) as npool:
            z = npool.tile([F, rpc], f32)
            nc.gpsimd.memset(z[:, 0:1], 0.0)
            nc.sync.dma_start(outT[:, :], z[:])
        return

    if packed:
        adjTp = nc.dram_tensor(
            "adjT", [n // (128 * jpd), 128, jpd * rpc], fp8,
            kind="ExternalInput",
        ).ap()
    else:
        adjT = nc.dram_tensor("adjT", [n, rpc], fp8, kind="ExternalInput").ap()
    moleT = nc.dram_tensor("moleT", [C, n], f16, kind="ExternalInput").ap()
    Waug = nc.dram_tensor("Waug", [C, F1], f16, kind="ExternalInput").ap()
    b2r = nc.dram_tensor("b2r16", [128, SEC * F], f32,
                         kind="ExternalInput").ap()
    outT = nc.dram_tensor("outT", [F, rpc], f32, kind="ExternalOutput").ap()

    with (
        tc.tile_pool(name="const", bufs=1) as const,
        tc.tile_pool(name="preps", bufs=4, space="PSUM") as pre_ps,
        tc.tile_pool(name="sml", bufs=3) as sml,
        tc.tile_pool(name="abf", bufs=abf_bufs) as abfp,
        tc.tile_pool(name="mainps", bufs=1, space="PSUM") as main_ps,
        tc.tile_pool(name="bcps", bufs=2, space="PSUM") as bc_ps,
        tc.tile_pool(name="epi", bufs=epi_bufs) as epi,
    ):
        full_ctx = (
            tc.For_i(0, hw_full, name="fullrep") if hw_full > 1 else None
        )
        for _it in range(full):
          with full_ctx if full_ctx is not None else _nullctx():
            hdt = fp8 if dr else f16
            # DoubleRow weight APs need a 16-byte-aligned pair stride: pad
            # the per-chunk H2 stride from 65 to 80 fp8 elements
            F1P = 80 if dr else F1
            # constants first on the ACT ring so the pre-pass isn't gated
            # behind the mole stream
            W_sb = const.tile([C, F1], f16)
            nc.scalar.dma_start(W_sb[:], Waug)
            b2_sb = const.tile([128, SEC * F], f32)
            nc.scalar.dma_start(b2_sb[:], b2r)
            moleT_sb = const.tile([C, n], f16)
            if no_pre or no_mole:  # timing probes only: skip the mole load
                nc.gpsimd.memset(moleT_sb[:, 0:128], 0.0)
            else:
                # split 0 on the SP ring AHEAD of the adj tiles (ring FIFO
                # guarantees it lands first); splits 1..3 on the ACT ring
                # so they stream concurrently with adj
                for s in range(NSEC):
                    sl = slice(s * (n // NSEC), (s + 1) * (n // NSEC))
                    eng = nc.sync if (s == 0 and interleave) else nc.scalar
                    eng.dma_start(moleT_sb[:, sl], moleT[:, sl])
            H2 = const.tile([128, jt * F1P], hdt)
            ones_sb = const.tile([1, F], f32)
            nc.gpsimd.memset(ones_sb[:], 1.0)

            h2v = H2[:].rearrange("p (c f) -> p c f", f=F1P)

            pss = [
                main_ps.tile([F1, 512], f32, name=f"mps{sb}", tag=f"mps{sb}")
                for sb in range(nsb)
            ]

            def pre_section(s):
                """Wh, s2, ev for chunks [s*SEC, (s+1)*SEC) -> H2 section.

                DVE volume is the scarce resource: the relu*b2 STT reads Wh
                straight from psum (no staging copy), the reduce runs on its
                fp16 output, and the H2 G-part is per-group STTs of psum
                against a materialised ev expansion (broadcast APs combine
                only with contiguous outputs, so ev is expanded once).
                """
                swc = sml.tile([128, SEC], f32, tag="swc")
                whc = sml.tile([128, SEC * F], f32, tag="whc")
                for g in range(SEC // G):
                    ps = pre_ps.tile([128, G * F1], f32)
                    for q in range(G):
                        cc = s * SEC + g * G + q
                        # [128(i), 65] = moleT[:, chunk].T @ [W | 0.2*W@b2]
                        nc.tensor.matmul(
                            ps[:, q * F1:(q + 1) * F1],
                            lhsT=moleT_sb[:, cc * 128:(cc + 1) * 128],
                            rhs=W_sb[:],
                            start=True,
                            stop=True,
                        )
                    ps3 = ps[:].rearrange("p (g f) -> p g f", f=F1)
                    # extract Wh from psum exactly ONCE (DVE psum reads run
                    # 2.2x slower than SBUF reads); everything downstream
                    # reads the SBUF copy
                    nc.vector.tensor_copy(
                        whc[:, g * G * F:(g + 1) * G * F].rearrange(
                            "p (g f) -> p g f", f=F),
                        ps3[:, :, 0:F],
                    )
                    nc.scalar.copy(swc[:, g * G:(g + 1) * G], ps3[:, :, F])
                m16 = sml.tile([128, SEC * F], f16, tag="m16")
                # m = relu(Wh) * (0.8*b2), fp16 out
                nc.vector.scalar_tensor_tensor(
                    m16[:], whc[:], 0.0, b2_sb[:], op0=ALU.max, op1=ALU.mult,
                )
                sr = sml.tile([128, SEC], f32, tag="sr")
                nc.vector.tensor_reduce(
                    sr[:], m16[:].rearrange("p (c f) -> p c f", f=F),
                    axis=mybir.AxisListType.X, op=ALU.add,
                )
                s2s = sml.tile([128, SEC], f32, tag="s2s")
                nc.vector.tensor_add(s2s[:], swc[:], sr[:])
                ev = sml.tile([128, SEC], f32, tag="ev")
                nc.scalar.activation(ev[:], s2s[:], AF.Exp)
                evx = sml.tile([128, SEC * F], f16, tag="evx")
                nc.vector.tensor_copy(
                    evx[:].rearrange("p (c f) -> p c f", f=F),
                    ev[:].rearrange("p c -> p c ()").broadcast_to(
                        [128, SEC, F]),
                )
                # H2 G-part: Wh * ev, SBUF-only, strided fp8 out (legal:
                # no broadcast AP involved)
                nc.vector.tensor_mul(
                    h2v[:, s * SEC:(s + 1) * SEC, 0:F],
                    whc[:].rearrange("p (c f) -> p c f", f=F),
                    evx[:].rearrange("p (c f) -> p c f", f=F),
                )
                nc.vector.tensor_copy(
                    h2v[:, s * SEC:(s + 1) * SEC, F:F1], ev[:]
                )

            def main_mms(abf, w, c0, sb_outer=False):
                """Main matmuls consuming adj chunks [c0, c0+w)."""
                if dr:
                    # fp8 DoubleRow: one matmul consumes a PAIR of j-chunks
                    abf3 = abf[:].rearrange("p (c r) -> p c r", c=w)
                    iters = ([(h, sb) for sb in range(nsb)
                              for h in range(0, w, 2)] if sb_outer else
                             [(h, sb) for h in range(0, w, 2)
                              for sb in range(nsb)])
                    for h, sb in iters:
                        if True:
                            jc = c0 + h
                            nc.tensor.matmul(
                                pss[sb][:],
                                lhsT=h2v[:, jc:jc + 2, 0:F1],
                                rhs=abf3[:, h:h + 2,
                                         sb * 512:(sb + 1) * 512],
                                start=(jc == 0),
                                stop=(jc == jt - 2),
                                perf_mode=mybir.MatmulPerfMode.DoubleRow,
                            )
                else:
                    for h in range(w):
                        jc = c0 + h
                        for sb in range(nsb):
                            nc.tensor.matmul(
                                pss[sb][:],
                                lhsT=h2v[:, 0:1, 0:F1] if same_w
                                else h2v[:, jc:jc + 1, 0:F1],
                                rhs=abf[:, h * rpc + sb * 512:
                                        h * rpc + (sb + 1) * 512],
                                start=(jc == 0),
                                stop=(jc == jt - 1),
                            )

            if interleave:
                # adj tile DMAs issued upfront on the SP ring (optionally in
                # dma_split sub-pieces so the first matmuls start earlier)
                abf_tiles = []
                for jd in range(NSEC):
                    abf = abfp.tile([128, jpd * rpc], fp8, name="abf",
                                    tag="abf")
                    a3 = abf[:].rearrange("p (c r) -> p c r", c=jpd)
                    for piece in range(dma_split):
                        cl = slice(piece * (jpd // dma_split),
                                   (piece + 1) * (jpd // dma_split))
                        nc.sync.dma_start(a3[:, cl, :], adjTp[jd, :, :]
                                          .rearrange("p (c r) -> p c r",
                                                     c=jpd)[:, cl, :])
                    abf_tiles.append(abf)
                for s in range(NSEC):
                    pre_section(s)
                    main_mms(abf_tiles[s], SEC, s * SEC,
                             sb_outer=(s == NSEC - 1))
            else:
                # measurement path: pre-pass fully first, then the main
                # pass (optionally looped) — matches the old structure
                if no_pre:
                    nc.gpsimd.memset(H2[:], 0.0)
                else:
                    for s in range(NSEC):
                        pre_section(s)
                adjT3 = (None if packed
                         else adjT.rearrange("(c p) r -> c p r", p=128))
                prime = (2, 2, 4) if dr else (1, 1, 2)
                widths = []
                if (not packed and jt > sum(prime)
                        and (jt - sum(prime)) % jpd == 0):
                    widths = list(prime)
                while sum(widths) < jt:
                    widths.append(jpd)
                if probe == "pe_only":
                    abf_c = const.tile([128, jpd * rpc], fp8, tag="abfc")
                    nc.vector.memset(abf_c[:], 0.0)
                rep_ctx = (
                    tc.For_i(0, hw_repeat, name="mainrep")
                    if hw_repeat > 1 else None
                )
                for rep in range(repeat):
                  with rep_ctx if rep_ctx is not None else _nullctx():
                    c0 = 0
                    for jd, w in enumerate(widths):
                        if packed:
                            src = adjTp[c0 // jpd]
                        else:
                            src = adjT3[c0:c0 + w, :, :].rearrange(
                                "c p r -> p c r")
                        if probe == "pe_only":
                            abf = abf_c
                        else:
                            abf = abfp.tile([128, w * rpc], fp8, name="abf",
                                            tag="abf")
                            dma_eng = (nc.scalar if (dma_alt and jd % 2)
                                       else nc.sync)
                            dma_eng.dma_start(
                                abf[:].rearrange("p (c r) -> p c r", c=w),
                                src,
                            )
                        if probe == "dma_only":
                            c0 += w
                            continue
                        main_mms(abf, w, c0)
                        c0 += w

            # ---- epilogue: out = elu(num / den), stored transposed ----
            if probe == "no_epi":
                for sb in range(nsb):
                    t = epi.tile([F1, 1], f32, tag=f"ne{sb}")
                    nc.vector.tensor_copy(t[:], pss[sb][:, 0:1])
                dz = epi.tile([F, rpc], f32, tag="dz")
                nc.gpsimd.memset(dz[:, 0:1], 0.0)
                nc.sync.dma_start(outT[:, :], dz[:])
                continue
            if probe:
                dz = epi.tile([F, rpc], f32, tag="dz")
                nc.gpsimd.memset(dz[:, 0:1], 0.0)
                nc.sync.dma_start(outT[:, :], dz[:])
                continue
            # per superblock: parallel num copy plus rec->bc (PE broadcast
            # of 1/den); the elu tail runs per superblock at [64, 512]
            # (full DVE/ACT rate) with both halves landing in one output
            # tile for a single 256KB store
            o = epi.tile([F, rpc], f32, tag="o")
            for sb in range(nsb):
                ps = pss[sb]
                numc = epi.tile([F, 512], f32, tag=f"numc{sb}")
                nc.vector.tensor_copy(numc[:], ps[0:F, :])
                rec = epi.tile([1, 512], f32, tag=f"rec{sb}")
                nc.vector.reciprocal(rec[:], ps[F:F1, :])
                bc = bc_ps.tile([F, 512], f32)
                nc.tensor.matmul(bc[:], lhsT=ones_sb[:], rhs=rec[:],
                                 start=True, stop=True)
                x = epi.tile([F, 512], f32, tag=f"x{sb}")
                nc.vector.tensor_mul(x[:], numc[:], bc[:])
                mneg = epi.tile([F, 512], f32, tag=f"mneg{sb}")
                nc.vector.tensor_scalar_min(mneg[:], x[:], 0.0)
                e = epi.tile([F, 512], f32, tag=f"e{sb}")
                nc.scalar.activation(e[:], mneg[:], AF.Exp)
                r = epi.tile([F, 512], f32, tag=f"r{sb}")
                nc.vector.tensor_scalar_max(r[:], x[:], 0.0)
                # o = (e + (-1)) + r  == elu(x)
                nc.vector.scalar_tensor_tensor(
                    o[:, sb * 512:(sb + 1) * 512], e[:], -1.0, r[:],
                    op0=ALU.add, op1=ALU.add,
                )
            nc.sync.dma_start(outT[:, :], o[:])


_CACHE = {}


def _build(n=N, rpc=RPC, repeat=1, abf_bufs=4, jpd=16, swdge_queues=1,
           no_pre=False, epi_bufs=2, mole_splits=4, packed=True,
           same_w=False, nop=False, dr=True, no_mole=False, full=1,
           hw_repeat=1, hw_full=1, probe=None, dma_alt=False,
           interleave=True, dma_split=2):
    key = (n, rpc, repeat, abf_bufs, jpd, swdge_queues, no_pre, epi_bufs,
           mole_splits, packed, same_w, nop, dr, no_mole, full, hw_repeat,
           hw_full, probe, dma_alt, interleave, dma_split)
    if key not in _CACHE:
        nc = bacc.Bacc(
            "TRN2", target_bir_lowering=False, debug=False, num_devices=NCORES,
            num_swdge_queues=swdge_queues,
        )
        with tile.TileContext(nc) as tc:
            _emit(tc, n, rpc, repeat, abf_bufs, jpd, no_pre, epi_bufs,
                  mole_splits, packed, same_w, nop, dr, no_mole, full,
                  hw_repeat, hw_full, probe, dma_alt, interleave, dma_split)
        nc.compile()
        _CACHE[key] = nc
    return _CACHE[key]


def _host_prep(mole_out, adj, W, b, n=N, rpc=RPC, ncores=NCORES,
               packed=True, jpd=16):
    mole_out = np.asarray(mole_out, dtype=np.float32)
    adj = np.asarray(adj)
    W = np.asarray(W, dtype=np.float32)
    b = np.asarray(b, dtype=np.float32)
    b2 = b[F:]
    moleT = np.ascontiguousarray(mole_out.T.astype(np.float16))  # [128, n]
    Waug = np.concatenate([W, (ALPHA * (W @ b2))[:, None]], axis=1)
    Waug = np.ascontiguousarray(Waug.astype(np.float16))         # [128, 65]
    b2r16 = np.tile(((1.0 - ALPHA) * b2).astype(np.float32), (128, 16))
    b2r16 = np.ascontiguousarray(b2r16)                          # [128, 1024]
    # adjacency as fp8 {0.0, 1.0}: 1.0 in e4m3 is byte 0x38
    adj8 = (np.asarray(adj, dtype=np.uint8) * np.uint8(0x38)).view(FP8_NP)
    in_maps = []
    for k in range(ncores):
        adjTk = np.ascontiguousarray(adj8[k * rpc:(k + 1) * rpc, :].T)
        if packed:
            adjTk = np.ascontiguousarray(
                adjTk.reshape(n // (128 * jpd), jpd, 128, rpc)
                .transpose(0, 2, 1, 3)
                .reshape(n // (128 * jpd), 128, jpd * rpc)
            )
        in_maps.append(
            {"adjT": adjTk, "moleT": moleT, "Waug": Waug, "b2r16": b2r16}
        )
    return in_maps


def _run(inputs, trace=False, build_kw=None, **kw):
    bkw = dict(build_kw or {})
    nc = _build(**bkw)
    in_maps = _host_prep(**inputs, packed=bkw.get("packed", True),
                         jpd=bkw.get("jpd", 16))
    res = run_bass_kernel_spmd(
        nc, in_maps, core_ids=list(range(NCORES)), trace=trace, **kw
    )
    out = np.concatenate([r["outT"].T for r in res.results], axis=0)
    return np.ascontiguousarray(out, dtype=np.float32), res


def kernel(mole_out, adj, W, b):
    out, _ = _run(dict(mole_out=mole_out, adj=adj, W=W, b=b))
    return out
